# revision 24
# baseline (speedup 1.0000x reference)
"""Trainium2 Bass kernel for nn_BasicAttentionBlock.

Data-parallel over batch: B=8 -> one batch element per NeuronCore.
Everything is fused on-chip; the attention tensor never touches HBM.

Relative-position logits are folded into the QK^T matmul via an augmented
contraction: per head, q_hat = [q (8); rw_shift (32); rh_shift (32)] and
k_hat = [k (8); onehot(x2) (32); onehot(y2) (32)], so that
q_hat . k_hat = q.k + q.rel_w[x2-x+31] + q.rel_h[y2-y+31] = full logits.

Scores are materialized per (head, key-block) as [128 keys, 1024 queries]
tiles in PSUM; exp runs on the scalar engine; the PV matmul uses a
stationary [v^T | ones] so the softmax denominator accumulates alongside
the numerator (normalization is applied after PV; softmax max-subtraction
is unnecessary since the logits are bounded).

All inputs ship in one DRAM blob (single DMA semaphore — the ISA allows
few sync-waits per instruction) and phase boundaries use all-engine
barriers to collapse DMA fan-in.
"""

import numpy as np
from contextlib import ExitStack

import concourse.bass as bass
import concourse.tile as tile
from concourse import bacc
from concourse import mybir
from concourse.bass_utils import run_bass_kernel_spmd

F32 = mybir.dt.float32
F32R = mybir.dt.float32r

NH = 8
C = 128
H = 32
W = 32
HW = 1024
DK = 64
DV = 64
DKH = 8
DVH = 8
OUT = 128
EPS = 1e-5

# blob column layout
BX = 0                      # x           [128, 1024]
BWQK = 1024                 # wqk         [128, 128]
BWV = BWQK + 128            # wv          [128, 64]
BWCO = BWV + 64             # wco         [128, 15*64]
BW2 = BWCO + 15 * 64        # w2          [128, 15*128]
BWAT = BW2 + 15 * 128       # wattn       [64, 64]
BRELW = BWAT + 64           # relw        [8, 63]
BRELH = BRELW + 63          # relh        [8, 63]
BCV = BRELH + 63            # cvec        [128, 5] = s1,t1,bqk,s2,t2p
BZ = BCV + 5                # zeros       [128, 128] (pad-zero DMA source)
BPV = BZ + 128              # pv pattern  [128, 2560] = (jb,h,40) ones block
BK1 = BPV + 2560            # blob part 1 ends here
BKC = BK1                   # kconst      [rows 8:72, 8192]
BTOT = BKC + NH * HW


def r(ap):
    return ap.bitcast(F32R)


def f(ap):
    return ap.bitcast(F32)


def build_program():
    nc = bacc.Bacc()

    blob_d = nc.dram_tensor("blob", [C, BTOT], F32R, kind="ExternalInput")
    y_d = nc.dram_tensor("y", [OUT, HW], F32, kind="ExternalOutput")

    with tile.TileContext(nc) as tc, ExitStack() as ctx:
        singles = ctx.enter_context(tc.tile_pool(name="singles", bufs=1))
        psmm = ctx.enter_context(tc.tile_pool(name="psmm", bufs=1, space="PSUM"))

        blob = singles.tile([C, BTOT], F32R)
        nc.sync.dma_start(out=blob[:, 0:BK1], in_=blob_d[:, 0:BK1])
        nc.sync.dma_start(out=blob[8:72, BKC:BTOT], in_=blob_d[8:72, BKC:BTOT])

        x_sb = f(blob[:, BX:BX + HW])
        wqk_sb = blob[:, BWQK:BWQK + 128]
        wv_sb = blob[:, BWV:BWV + 64]
        wco_sb = blob[:, BWCO:BWCO + 15 * 64]
        w2_sb = blob[:, BW2:BW2 + 15 * 128]
        wattn_sb = blob[0:64, BWAT:BWAT + 64]
        relw_sb = blob[0:8, BRELW:BRELW + 63]
        relh_sb = blob[0:8, BRELH:BRELH + 63]
        s1_sb = f(blob[:, BCV:BCV + 1])
        t1_sb = f(blob[:, BCV + 1:BCV + 2])
        bqk_sb = f(blob[:, BCV + 2:BCV + 3])
        s2_sb = f(blob[:, BCV + 3:BCV + 4])
        t2p_sb = f(blob[:, BCV + 4:BCV + 5])
        # KHAT rows 0:8 = k_h (device-written), 8:40 = onehot(x2),
        # 40:72 = onehot(y2) (upload); shares storage with the blob.
        khat = blob[0:72, BKC:BTOT]

        # QHAT rows: 0:8 = q_h, 8:40 = shifted Aw, 40:72 = shifted Ah
        qhat_t = singles.tile([72, NH * HW], F32R)
        qhat = qhat_t[:]

        # Activations feeding 3x3 convs are stored row-padded: 36 rows of 32,
        # real row y at offset (y+2)*32, rows 0,1,34,35 are zeros. Conv taps
        # then read single contiguous 512-element runs (walrus requires one
        # free dim on matmul RHS); the x-edge wraparound into the adjacent
        # row is cancelled by small correction matmuls with negated weights.
        PAD = 36 * 32
        AOFF = 64  # offset of real (0,0)

        def interior(ap):
            return ap[:, AOFF:AOFF + HW]

        bzero2 = blob[:, BZ:BZ + 128].rearrange("c (a b) -> c a b", a=2)

        def zero_pads(t):
            ap = t[:]
            pad_ap = bass.AP(tensor=ap.tensor, offset=ap.offset,
                             ap=[ap.ap[0], [1088, 2], [1, 64]])
            nc.sync.dma_start(out=pad_ap, in_=bzero2)

        # ---- act1 = relu(s1 * x + t1) ----
        act1 = singles.tile([C, PAD], F32R)
        zero_pads(act1)
        nc.scalar.activation(
            r(interior(act1[:])), x_sb,
            mybir.ActivationFunctionType.Relu,
            bias=t1_sb, scale=s1_sb,
        )
        a1i = interior(act1[:])

        # ---- q/k: psum_qk[0:64]=q (scaled), [64:128]=k ----
        psum_qk = psmm.tile([128, HW], F32, tag="mm")
        for half in range(2):
            nc.tensor.matmul(
                psum_qk[:, half * 512:(half + 1) * 512],
                r(wqk_sb),
                r(a1i[:, half * 512:(half + 1) * 512]),
                start=True, stop=True,
            )
        # add per-channel bias, then scatter q/k rows into QHAT/KHAT head
        # slices via DMA (engine ops need quadrant-aligned start partitions)
        ctx_rel = tc.tile_pool(name="relp", bufs=1)
        relp = ctx_rel.__enter__()
        qk_sb = relp.tile([128, HW], F32R)
        nc.vector.tensor_scalar_add(r(qk_sb[:]), psum_qk[:], bqk_sb)
        for h in range(NH):
            nc.sync.dma_start(
                out=qhat[0:8, h * HW:(h + 1) * HW],
                in_=qk_sb[h * 8:(h + 1) * 8, :],
            )
            nc.sync.dma_start(
                out=khat[0:8, h * HW:(h + 1) * HW],
                in_=qk_sb[64 + h * 8:64 + (h + 1) * 8, :],
            )

        # ---- v^T with ones columns for the denominator ----
        # pv_lhsT free layout: (jb 8, h 8, 40) = [8 v-cols | 24 pad | 8 ones]
        # (pad keeps the denominator rows at PSUM partition 32, a legal
        # engine start partition)
        pv_lhsT = blob[:, BPV:BPV + 2560]
        pv_view = pv_lhsT.rearrange("p (jb h c) -> p jb h c", jb=8, h=8)
        for pb in range(8):
            psum_vt = psmm.tile([128, DV], F32, tag="mm")
            nc.tensor.matmul(
                psum_vt[:],
                r(a1i[:, pb * 128:(pb + 1) * 128]),
                r(wv_sb),
                start=True, stop=True,
            )
            nc.vector.tensor_copy(
                r(pv_view[:, pb, :, 0:8]),
                psum_vt[:].rearrange("p (h c) -> p h c", h=8),
            )

        # ---- relative logits: AwT/AhT = rel^T @ q_h, then shifted into QHAT
        awt = relp.tile([63, NH * HW], F32R)
        aht = relp.tile([63, NH * HW], F32R)
        for h in range(NH):
            for rel_sb, dst in ((relw_sb, awt), (relh_sb, aht)):
                psum_rel = psmm.tile([63, HW], F32, tag="mm")
                for half in range(2):
                    nc.tensor.matmul(
                        psum_rel[:, half * 512:(half + 1) * 512],
                        r(rel_sb),
                        r(qhat[0:8, h * HW + half * 512:h * HW + (half + 1) * 512]),
                        start=True, stop=True,
                    )
                if dst is awt:
                    nc.scalar.copy(r(dst[:, h * HW:(h + 1) * HW]), psum_rel[:])
                else:
                    nc.vector.tensor_copy(r(dst[:, h * HW:(h + 1) * HW]), psum_rel[:])
        # shift: QHAT[8+x2', i=(y,x)] = AwT[x2' - x + 31, i]  (window DMA per x)
        for src, prow in ((awt, 8), (aht, 40)):
            sview = src[:].rearrange("m (h y x) -> m h y x", h=8, x=W)
            dview = qhat[prow:prow + 32, :].rearrange("m (h y x) -> m h y x", h=8, x=W)
            for xx in range(W):
                nc.sync.dma_start(
                    out=dview[:, :, :, xx],
                    in_=sview[31 - xx:63 - xx, :, :, xx],
                )
        ctx_rel.__exit__(None, None, None)

        # collapse the 80-DMA fan-in before the attention phase
        tc.strict_bb_all_engine_barrier()

        # ---- attention ----
        work = ctx.enter_context(tc.tile_pool(name="work", bufs=2))
        nrm = ctx.enter_context(tc.tile_pool(name="nrm", bufs=1))
        ctx_pss = tc.tile_pool(name="pss", bufs=2, space="PSUM")
        pss = ctx_pss.__enter__()
        ctx_pspv = tc.tile_pool(name="pspv", bufs=1, space="PSUM")
        pspv = ctx_pspv.__enter__()
        attn_sb = singles.tile([DV, HW], F32R)
        for h in range(NH):
            psum_pv = pspv.tile([40, HW], F32, tag="pv")
            for jb in range(8):
                psum_s = pss.tile([128, HW], F32)
                for half in range(2):
                    nc.tensor.matmul(
                        psum_s[:, half * 512:(half + 1) * 512],
                        r(khat[:, h * HW + jb * 128:h * HW + (jb + 1) * 128]),
                        r(qhat[:, h * HW + half * 512:h * HW + (half + 1) * 512]),
                        start=True, stop=True,
                    )
                e_sb = work.tile([128, HW], F32R, tag="esb")
                nc.scalar.activation(
                    r(e_sb[:]), psum_s[:], mybir.ActivationFunctionType.Exp,
                )
                for half in range(2):
                    nc.tensor.matmul(
                        psum_pv[:, half * 512:(half + 1) * 512],
                        r(pv_lhsT[:, (jb * 8 + h) * 40:(jb * 8 + h + 1) * 40]),
                        r(e_sb[:, half * 512:(half + 1) * 512]),
                        start=(jb == 0), stop=(jb == 7),
                    )
            # normalize: attn = num * 1/den. TensorTensor operands must sit on
            # identical partitions, so move 1/den from partitions 32:40 down
            # to 0:8 with a DMA before the multiply.
            den40 = nrm.tile([40, HW], F32, tag="den40")
            nc.vector.reciprocal(den40[32:40, :], psum_pv[32:40, :])
            den0 = nrm.tile([8, HW], F32, tag="den0")
            nc.sync.dma_start(out=den0[:], in_=den40[32:40, :])
            attn_tmp = nrm.tile([8, HW], F32R, tag="atmp")
            nc.vector.tensor_mul(r(attn_tmp[:]), psum_pv[0:8, :], den0[:])
            nc.sync.dma_start(
                out=attn_sb[h * 8:(h + 1) * 8, :], in_=attn_tmp[:],
            )
        ctx_pspv.__exit__(None, None, None)
        ctx_pss.__exit__(None, None, None)
        fixp = ctx.enter_context(tc.tile_pool(name="fixp", bufs=1, space="PSUM"))

        # collapse the 8 attn-gather DMAs before the projection
        tc.strict_bb_all_engine_barrier()

        # ---- attn projection (1x1) -> act2 rows 64:128 ----
        act2 = singles.tile([OUT, PAD], F32R)
        zero_pads(act2)
        psum_ap = psmm.tile([DV, HW], F32, tag="mm")
        for half in range(2):
            nc.tensor.matmul(
                psum_ap[:, half * 512:(half + 1) * 512],
                r(wattn_sb),
                r(attn_sb[:, half * 512:(half + 1) * 512]),
                start=True, stop=True,
            )
        nc.scalar.activation(
            r(interior(act2[64:128, :])),
            psum_ap[:],
            mybir.ActivationFunctionType.Relu,
            bias=t2p_sb[64:128, :], scale=s2_sb[64:128, :],
        )

        # ---- 3x3 convs: 9 shifted contiguous matmuls + 6 edge corrections
        def conv3x3(dst_psum, src_pad, w_sb, mout):
            for t in range(9):
                dy, dx = t // 3 - 1, t % 3 - 1
                for (ya, yb) in ((0, 16), (16, 32)):
                    nc.tensor.matmul(
                        dst_psum[:, ya * 32:yb * 32],
                        r(w_sb[:, t * mout:(t + 1) * mout]),
                        r(src_pad[:, AOFF + (ya + dy) * 32 + dx:
                                  AOFF + (ya + dy) * 32 + dx + 512]),
                        start=(t == 0), stop=(t == 8),
                    )
            # cancel the x-wraparound: tap (dy,dx=+1) wrongly added
            # W @ src[y+dy+1, 0] at out (y,31); (dy,dx=-1) added
            # W @ src[y+dy-1, 31] at out (y,0). Weight slots 9..14 hold -W.
            # Corrections land in a contiguous side tile (matmul outputs
            # must be contiguous), then DVE-add into the main accumulator.
            corr = [(dy, dx) for dy in (-1, 0, 1) for dx in (-1, 1)]
            psum_fix = fixp.tile([mout, 192], F32, tag="fix")
            for ci, (dy, dx) in enumerate(corr):
                t = 9 + ci
                xo = 31 if dx == 1 else 0
                src_off = AOFF + (dy + dx) * 32 + xo - dx * 31
                nc.tensor.matmul(
                    psum_fix[:, ci * 32:(ci + 1) * 32],
                    r(w_sb[:, t * mout:(t + 1) * mout]),
                    r(src_pad[:, src_off:src_off + HW]
                      .rearrange("c (y x) -> c y x", x=W)[:, :, 0]),
                    start=True, stop=True,
                )
            fix_sb = work.tile([mout, 192], F32, tag="fixsb")
            nc.vector.tensor_copy(fix_sb[:], psum_fix[:])
            dview = dst_psum[:].rearrange("c (y x) -> c y x", x=W)
            for ci, (dy, dx) in enumerate(corr):
                xo = 31 if dx == 1 else 0
                nc.vector.tensor_add(
                    dview[:, :, xo], dview[:, :, xo],
                    fix_sb[:, ci * 32:(ci + 1) * 32],
                )

        psum_co = psmm.tile([64, HW], F32, tag="mm")
        conv3x3(psum_co, act1[:], wco_sb, 64)
        nc.scalar.activation(
            r(interior(act2[0:64, :])),
            psum_co[:],
            mybir.ActivationFunctionType.Relu,
            bias=t2p_sb[0:64, :], scale=s2_sb[0:64, :],
        )

        psum_out = psmm.tile([OUT, HW], F32, tag="mm")
        conv3x3(psum_out, act2[:], w2_sb, OUT)

        y_sb = singles.tile([OUT, HW], F32)
        nc.vector.tensor_add(y_sb[:], psum_out[:], x_sb)
        nc.sync.dma_start(out=y_d[:], in_=y_sb[:])

    nc.finalize()
    return nc


def prep_inputs(inputs):
    """Host-side folding of bn/bias constants into one blob (minus x)."""
    s1 = inputs["bn1_g"] / np.sqrt(inputs["bn1_v"] + EPS)
    t1 = inputs["bn1_b"] - inputs["bn1_m"] * s1

    qkv_w = inputs["qkv_w"][:, :, 0, 0]  # [192, 128]
    qkv_b = inputs["qkv_b"]
    sc = DKH ** -0.5
    wqk = np.concatenate([qkv_w[0:DK] * sc, qkv_w[DK:2 * DK]], axis=0).T
    bqk = np.concatenate([qkv_b[0:DK] * sc, qkv_b[DK:2 * DK]])
    wv = qkv_w[2 * DK:].T

    def conv_taps(w):
        t = np.transpose(w, (2, 3, 1, 0)).reshape(9, w.shape[1], w.shape[0])
        neg = [-t[(dy + 1) * 3 + (dx + 1)]
               for dy in (-1, 0, 1) for dx in (-1, 1)]
        return np.concatenate([t, np.stack(neg)], axis=0)

    wco = conv_taps(inputs["convout_w"])    # [15, 128, 64]
    w2 = conv_taps(inputs["conv2_w"])       # [15, 128, 128]
    wattn = inputs["attn_w"][:, :, 0, 0].T

    s2 = inputs["bn2_g"] / np.sqrt(inputs["bn2_v"] + EPS)
    t2 = inputs["bn2_b"] - inputs["bn2_m"] * s2
    battn_eff = inputs["attn_w"][:, :, 0, 0] @ qkv_b[2 * DK:] + inputs["attn_b"]
    t2p = t2 + s2 * np.concatenate([inputs["convout_b"], battn_eff])

    oh = np.zeros((64, HW), np.float32)
    j = np.arange(HW)
    oh[j % W, j] = 1.0
    oh[32 + j // W, j] = 1.0
    kconst = np.tile(oh, (1, NH))

    blob = np.zeros((C, BTOT), np.float32)
    blob[:, BWQK:BWQK + 128] = wqk
    blob[:, BWV:BWV + 64] = wv
    blob[:, BWCO:BWCO + 15 * 64] = np.transpose(wco, (1, 0, 2)).reshape(C, -1)
    blob[:, BW2:BW2 + 15 * 128] = np.transpose(w2, (1, 0, 2)).reshape(C, -1)
    blob[0:64, BWAT:BWAT + 64] = wattn
    blob[0:8, BRELW:BRELW + 63] = inputs["rel_w"].T
    blob[0:8, BRELH:BRELH + 63] = inputs["rel_h"].T
    blob[:, BCV + 0] = s1
    blob[:, BCV + 1] = t1
    blob[:, BCV + 2] = bqk
    blob[:, BCV + 3] = s2
    blob[:, BCV + 4] = t2p
    pv = np.zeros((C, 8 * 8 * 40), np.float32)
    pv.reshape(C, 64, 40)[:, :, 32:40] = 1.0
    blob[:, BPV:BPV + 2560] = pv
    blob[8:72, BKC:BTOT] = kconst
    return blob


_PROGRAM = None


def get_program():
    global _PROGRAM
    if _PROGRAM is None:
        _PROGRAM = build_program()
    return _PROGRAM


def make_in_maps(inputs):
    blob = prep_inputs(inputs)
    x = np.asarray(inputs["x"], dtype=np.float32)
    maps = []
    for b in range(x.shape[0]):
        m = blob.copy()
        m[:, BX:BX + HW] = x[b].reshape(C, HW)
        maps.append({"blob": np.ascontiguousarray(m)})
    return maps


def kernel(**inputs):
    nc = get_program()
    in_maps = make_in_maps(inputs)
    res = run_bass_kernel_spmd(nc, in_maps, list(range(len(in_maps))))
    out = np.stack([m["y"].reshape(OUT, H, W) for m in res.results])
    return out.astype(np.float32)
